# Initial kernel scaffold
#
"""Bass/Tile MHA kernel for trn2 — builder + host shard/unshard helpers.

Per-core work (8 cores): core c handles batch b=c//2, head-group g=c%2
(8 of 16 heads). Head pairs share 128-partition tiles at bases 0 / 64 so
the K=48 QK^T matmuls land in distinct PE row-groups (free 2x packing),
and the AV matmuls use PSUM column groups 0 / 64 (col packing).

Dataflow (all matmuls bf16 in / fp32 PSUM accumulate):
  qkT[d_h, t]  = w_qk^T x           (lhsT=w_qk tile, rhs=x^T tile)
  V[t, d_v]    = x w_v              (lhsT=x^T tile, rhs=w_v)
  S^T[k, q]    = (K^T)^T Q^T        (K=48 contraction, row-packed pairs)
  P^T          = exp(S^T)           (ScalarE true exp / VectorE fast-exp,
                                     split tunable per kt)
  outT'[d,q],l = (V|1)^T P^T        (ones column gives softmax denoms)
  outT         = outT' * bcast(1/l) + b_v
  y[t, j]      = outT^T w_out       (+ b_out and cross-core sum on host)
"""

import math

import numpy as np
import ml_dtypes

import concourse.bass as bass
import concourse.mybir as mybir
import concourse.tile as tile
from concourse import bacc

F32 = mybir.dt.float32
BF16 = mybir.dt.bfloat16
I16 = mybir.dt.int16
AF = mybir.ActivationFunctionType
OP = mybir.AluOpType

DIM = 768
PH = 48
NP = 4          # head pairs per core
HC = 8          # heads per core
NDT = DIM // 128  # 6 contraction tiles for the projections

# Schraudolph fast-exp in bf16 bit space: bits = round(x*128/ln2 + (127*128 - C))
SCH_A = 128.0 / math.log(2.0)
SCH_C = 4.7
# +0.5: the fp32->int16 convert truncates, this re-centers it to round-nearest
SCH_B = 127.0 * 128.0 - SCH_C + 0.5


def build_kernel(T=2048, dve_slots=frozenset({1, 3, 5, 7, 9, 11, 13}),
                 num_devices=8, debug_taps=False):
    """Returns compiled Bacc module. dve_slots: which of 16 (kt*2+hh)%16
    pipeline slots run fast-exp on VectorE instead of exp on ScalarE."""
    KT = T // 128                 # k-tiles (token tiles)
    QCW = min(512, T)             # q chunk width (one PSUM bank)
    NQG = T // QCW                # q groups, one chunk each

    nc = bacc.Bacc("TRN2", target_bir_lowering=False, debug=False,
                   num_devices=num_devices)
    taps = {}
    if debug_taps:
        taps["qk"] = nc.dram_tensor("tap_qk", (128, NP * 2 * T), BF16, kind="ExternalOutput")
        taps["v"] = nc.dram_tensor("tap_v", (128, KT * HC * 65), BF16, kind="ExternalOutput")
        taps["pt"] = nc.dram_tensor("tap_pt", (128, QCW), BF16, kind="ExternalOutput")
        taps["av"] = nc.dram_tensor("tap_av", (128, QCW), F32, kind="ExternalOutput")
        taps["rbc"] = nc.dram_tensor("tap_rbc", (128, QCW), F32, kind="ExternalOutput")
        taps["outT"] = nc.dram_tensor("tap_outT", (128, NP * T), BF16, kind="ExternalOutput")

    xt_d = nc.dram_tensor("xt", (DIM, T), BF16, kind="ExternalInput")
    wqk_d = nc.dram_tensor("wqk", (DIM, NP * 2 * 128), BF16, kind="ExternalInput")
    wv_d = nc.dram_tensor("wv", (DIM, HC * PH), BF16, kind="ExternalInput")
    wo_d = nc.dram_tensor("wo", (NP * 128, DIM), BF16, kind="ExternalInput")
    bqk_d = nc.dram_tensor("bqk", (128, NP * 2), F32, kind="ExternalInput")
    y_d = nc.dram_tensor("y", (T, DIM), F32, kind="ExternalOutput")

    with tile.TileContext(nc) as tc:
        with (
            tc.tile_pool(name="const", bufs=1) as cpool,
            tc.tile_pool(name="pt", bufs=8) as ptpool,
            tc.tile_pool(name="norm", bufs=2) as npool,
            tc.tile_pool(name="ysb", bufs=2) as ypool,
            tc.tile_pool(name="st", bufs=4, space="PSUM") as stpool,
            tc.tile_pool(name="av", bufs=3, space="PSUM") as avpool,
            tc.tile_pool(name="pj", bufs=1, space="PSUM") as pjpool,
        ):
            # ---- persistent SBUF tensors ----
            xt_sb = cpool.tile([128, NDT, T], BF16, tag="xt")
            wqk_sb = cpool.tile([128, NDT, NP * 2 * 128], BF16, tag="wqk")
            wv_sb = cpool.tile([128, NDT, HC * PH], BF16, tag="wv")
            wo_sb = cpool.tile([128, NP, DIM], BF16, tag="wo")
            bqk_sb = cpool.tile([128, NP * 2], F32, tag="bqk")
            # per-head Q^T/K^T, rows 48-127 zeroed: padding the QK
            # contraction to K=128 keeps every matmul in plain 128x128 mode
            # (mode switches drain the PE and cost ~300ns each)
            qk_sb = cpool.tile([128, HC, 2, T], BF16, tag="qk")
            # V' columns per head: 0 = ones (softmax denominator lands on
            # PSUM row 0 / 64 of the shared bank), 1-48 = V, 49-63 = zero
            v_sb = cpool.tile([128, KT, HC, 64], BF16, tag="v")
            outT_sb = cpool.tile([128, NP, T], BF16, tag="outT")

            # ---- input DMAs ----
            for dt_i in range(NDT):
                nc.sync.dma_start(xt_sb[:, dt_i, :], xt_d[dt_i * 128:(dt_i + 1) * 128, :])
                nc.sync.dma_start(wqk_sb[:, dt_i, :], wqk_d[dt_i * 128:(dt_i + 1) * 128, :])
                nc.sync.dma_start(wv_sb[:, dt_i, :], wv_d[dt_i * 128:(dt_i + 1) * 128, :])
            for p in range(NP):
                nc.sync.dma_start(wo_sb[:, p, :], wo_d[p * 128:(p + 1) * 128, :])
            nc.sync.dma_start(bqk_sb[:], bqk_d[:])

            # ones column for the softmax-denominator trick; zero pads
            nc.gpsimd.memset(v_sb[:, :, :, 0:1], 1.0)
            nc.gpsimd.memset(v_sb[:, :, :, PH + 1:64], 0.0)
            nc.gpsimd.memset(qk_sb[32:64, :, :, :], 0.0)
            nc.gpsimd.memset(qk_sb[64:96, :, :, :], 0.0)
            nc.gpsimd.memset(qk_sb[96:128, :, :, :], 0.0)
            # pad rows (48-63, 112-127) must be finite; 32-aligned starts,
            # the real rows 32-47 / 96-111 are overwritten by normalize
            nc.gpsimd.memset(outT_sb[32:64, :, :], 0.0)
            nc.gpsimd.memset(outT_sb[96:128, :, :], 0.0)

            def qkT_proj(p):
                # qkT[d_h, t] for pair p: rows 0-47 head A dims, 64-111 head B
                for qk in range(2):
                    col0 = (p * 2 + qk) * 128
                    for tcI in range(T // QCW):
                        ps = pjpool.tile([128, QCW], F32, tag="pj")
                        for dt_i in range(NDT):
                            for mh in range(2):
                                nc.tensor.matmul(
                                    ps[mh * 64:(mh + 1) * 64, :],
                                    wqk_sb[:, dt_i, col0 + mh * 64:col0 + (mh + 1) * 64],
                                    xt_sb[:, dt_i, tcI * QCW:(tcI + 1) * QCW],
                                    start=(dt_i == 0), stop=(dt_i == NDT - 1),
                                    skip_group_check=True)
                        tsl = np.s_[tcI * QCW:(tcI + 1) * QCW]
                        nc.scalar.activation(
                            qk_sb[0:PH, p * 2, qk, tsl], ps[0:PH, :],
                            AF.Identity, bias=bqk_sb[0:PH, p * 2 + qk:p * 2 + qk + 1])
                        nc.scalar.activation(
                            qk_sb[0:PH, p * 2 + 1, qk, tsl], ps[64:64 + PH, :],
                            AF.Identity, bias=bqk_sb[64:64 + PH, p * 2 + qk:p * 2 + qk + 1])

            def v_proj():
                for tt in range(KT):
                    # full-bank tile so every pj slot stays bank-aligned
                    psb = pjpool.tile([128, 512], F32, tag="pj")
                    ps = psb[:, 0:HC * PH]
                    for dt_i in range(NDT):
                        for mh in range(2):
                            nc.tensor.matmul(
                                ps[mh * 64:(mh + 1) * 64, :],
                                xt_sb[:, dt_i, tt * 128 + mh * 64:tt * 128 + (mh + 1) * 64],
                                wv_sb[:, dt_i, :],
                                start=(dt_i == 0), stop=(dt_i == NDT - 1),
                                skip_group_check=True)
                    nc.scalar.activation(
                        v_sb[:, tt, :, 1:PH + 1],
                        ps[:].rearrange("p (h d) -> p h d", h=HC),
                        AF.Copy)

            def attention(p, qg):
                cs = np.s_[qg * QCW:(qg + 1) * QCW]
                av = avpool.tile([128, QCW], F32, tag="av", name="av")
                LEAD = 2  # AV trails QK by 2 k-tiles so exp latency hides
                pts = {}

                def qk_emit(kt):
                    for hh in range(2):
                        st = stpool.tile([128, QCW], F32, tag="st")
                        for mh in range(2):
                            nc.tensor.matmul(
                                st[mh * 64:(mh + 1) * 64, :],
                                qk_sb[:, p * 2 + hh, 1,
                                      kt * 128 + mh * 64:kt * 128 + (mh + 1) * 64],
                                qk_sb[:, p * 2 + hh, 0, cs],
                                start=True, stop=True,
                                skip_group_check=True)
                        pt = ptpool.tile([128, QCW], BF16, tag="pt")
                        if (kt * 2 + hh) % 16 in dve_slots:
                            nc.vector.tensor_scalar(
                                pt[:].bitcast(I16), st[:], SCH_A, SCH_B,
                                OP.mult, OP.add)
                        else:
                            nc.scalar.activation(pt[:], st[:], AF.Exp)
                        pts[(kt, hh)] = pt
                        if debug_taps and p == 0 and qg == 0 and kt == 0 and hh == 0:
                            nc.sync.dma_start(taps["pt"][:], pt[:])

                def av_emit(kt):
                    for hh in range(2):
                        nc.tensor.matmul(
                            av[hh * 64:(hh + 1) * 64, :],
                            v_sb[:, kt, p * 2 + hh, :],
                            pts.pop((kt, hh))[:],
                            start=(kt == 0), stop=(kt == KT - 1),
                            skip_group_check=True)

                for kt in range(KT + LEAD):
                    if kt < KT:
                        qk_emit(kt)
                    if kt >= LEAD:
                        av_emit(kt - LEAD)
                # normalize + bias into outT (denominators live in row 64).
                # partition_broadcast is only reliable with base-0 in/out APs,
                # so each head gets its own base-0 recip + broadcast tiles.
                r2a = npool.tile([128, QCW], F32, tag="r2", name="r2a")
                r2b = npool.tile([128, QCW], F32, tag="r2", name="r2b")
                rbca = npool.tile([128, QCW], F32, tag="rbc", name="rbca")
                rbcb = npool.tile([128, QCW], F32, tag="rbc", name="rbcb")
                lra = npool.tile([128, QCW], F32, tag="lr", name="lra")
                lrb = npool.tile([128, QCW], F32, tag="lr", name="lrb")
                nc.scalar.copy(lra[0:1, :], av[0:1, :])
                nc.scalar.copy(lrb[0:1, :], av[64:65, :])
                nc.vector.reciprocal_approx_fast(r2a[0:1, :], lra[0:1, :])
                nc.vector.reciprocal_approx_fast(r2b[0:1, :], lrb[0:1, :])
                nc.gpsimd.partition_broadcast(rbca[0:PH + 1, :], r2a[0:1, :])
                nc.gpsimd.partition_broadcast(rbcb[0:PH + 1, :], r2b[0:1, :])
                nc.vector.tensor_mul(outT_sb[0:PH + 1, p, cs],
                                     av[0:PH + 1, :], rbca[0:PH + 1, :])
                nc.vector.tensor_mul(outT_sb[64:64 + PH + 1, p, cs],
                                     av[64:64 + PH + 1, :], rbcb[0:PH + 1, :])
                if debug_taps and p == 0 and qg == 0:
                    avs = npool.tile([128, QCW], F32, tag="avs")
                    nc.vector.tensor_copy(avs[0:65, :], av[0:65, :])
                    nc.sync.dma_start(taps["av"][0:65, :], avs[0:65, :])
                    nc.sync.dma_start(taps["rbc"][0:PH, :], rbca[0:PH, :])
                    nc.sync.dma_start(taps["rbc"][64:64 + PH, :], rbcb[0:PH, :])

            def final_proj(qg):
                for tt in range(QCW // 128):
                    t0 = qg * QCW + tt * 128
                    ysb = ypool.tile([128, DIM], F32, tag="ysb")
                    for jc in range(2):
                        js = np.s_[jc * 384:(jc + 1) * 384]
                        psb = pjpool.tile([128, 512], F32, tag="pj", name=f"yp{jc}")
                        ps = psb[:, 0:384]
                        for p in range(NP):
                            for mh in range(2):
                                nc.tensor.matmul(
                                    ps[mh * 64:(mh + 1) * 64, :],
                                    outT_sb[:, p, t0 + mh * 64:t0 + (mh + 1) * 64],
                                    wo_sb[:, p, js],
                                    start=(p == 0), stop=(p == NP - 1),
                                    skip_group_check=True)
                        if (tt + jc) % 2 == 0:
                            nc.scalar.copy(ysb[:, js], ps[:])
                        else:
                            nc.vector.tensor_copy(ysb[:, js], ps[:])
                    nc.sync.dma_start(y_d[t0:t0 + 128, :], ysb[:])

            # ---- emission order (scheduling priority) ----
            qkT_proj(0)
            v_proj()
            for qg in range(NQG):
                for p in range(NP):
                    if qg == 0 and p + 1 < NP:
                        qkT_proj(p + 1)
                    attention(p, qg)
                final_proj(qg)

            if debug_taps:
                nc.sync.dma_start(
                    taps["qk"][:], qk_sb[:].rearrange("p a b t -> p (a b t)"))
                nc.sync.dma_start(
                    taps["v"][:], v_sb[:].rearrange("p a b c -> p (a b c)"))
                nc.sync.dma_start(
                    taps["outT"][:], outT_sb[:].rearrange("p a t -> p (a t)"))

    nc.compile()
    return nc


# ---------------- host-side sharding ----------------

def host_prep(x, w_in, b_in, w_out, T=2048):
    """Full inputs -> list of 8 per-core input dicts."""
    scale = 1.0 / math.sqrt(PH)
    wr = np.asarray(w_in).reshape(DIM, 16, 3, PH)
    br = np.asarray(b_in).reshape(16, 3, PH)
    wog = np.asarray(w_out)  # (768, 768), row dv = h*48+d
    in_maps = []
    for c in range(8):
        b, g = divmod(c, 2)
        wqk = np.zeros((DIM, NP * 2 * 128), np.float32)
        bqk = np.zeros((128, NP * 2), np.float32)
        wv = np.zeros((DIM, HC * PH), np.float32)
        wo = np.zeros((NP * 128, DIM), np.float32)
        for p in range(NP):
            for hh, base in ((0, 0), (1, 64)):
                gh = g * 8 + p * 2 + hh
                wqk[:, (p * 2) * 128 + base:(p * 2) * 128 + base + PH] = wr[:, gh, 0] * scale
                wqk[:, (p * 2 + 1) * 128 + base:(p * 2 + 1) * 128 + base + PH] = wr[:, gh, 1]
                bqk[base:base + PH, p * 2] = br[gh, 0] * scale
                bqk[base:base + PH, p * 2 + 1] = br[gh, 1]
                wv[:, (p * 2 + hh) * PH:(p * 2 + hh + 1) * PH] = wr[:, gh, 2]
                wo[p * 128 + base + 1:p * 128 + base + 1 + PH, :] = wog[gh * PH:(gh + 1) * PH, :]
        in_maps.append({
            "xt": np.ascontiguousarray(np.asarray(x)[b].T).astype(ml_dtypes.bfloat16),
            "wqk": wqk.astype(ml_dtypes.bfloat16),
            "wv": wv.astype(ml_dtypes.bfloat16),
            "wo": wo.astype(ml_dtypes.bfloat16),
            "bqk": bqk,
        })
    return in_maps


def host_post(results, b_out, b_in, w_out, B=4, T=2048):
    # the V bias contributes bv @ w_out, a per-column constant: add on host
    bv_all = np.asarray(b_in).reshape(16, 3, PH)[:, 2, :].reshape(DIM)
    const = np.asarray(b_out) + bv_all @ np.asarray(w_out)
    out = np.empty((B, T, DIM), np.float32)
    for b in range(B):
        out[b] = results[2 * b]["y"] + results[2 * b + 1]["y"] + const[None, :]
    return out


# ---------------- self-contained kernel() entry point ----------------

_CACHED = {}


def _get_nc():
    if "nc" not in _CACHED:
        _CACHED["nc"] = build_kernel(T=2048, num_devices=8)
    return _CACHED["nc"]


def kernel(x, w_in, b_in, w_out, b_out):
    """Full-input MHA forward on 8 NeuronCores.

    x: (4, 2048, 768) f32; w_in: (768, 2304); b_in: (2304,);
    w_out: (768, 768); b_out: (768,). Returns (4, 2048, 768) f32.
    """
    from concourse.bass_utils import run_bass_kernel_spmd

    x = np.asarray(x, np.float32)
    w_in = np.asarray(w_in, np.float32)
    b_in = np.asarray(b_in, np.float32)
    w_out = np.asarray(w_out, np.float32)
    b_out = np.asarray(b_out, np.float32)

    nc = _get_nc()
    in_maps = host_prep(x, w_in, b_in, w_out, T=2048)
    res = run_bass_kernel_spmd(nc, in_maps, core_ids=list(range(8)))
    return host_post(res.results, b_out, b_in, w_out, B=4, T=2048)



# revision 1
# speedup vs baseline: 1.1833x; 1.1833x over previous
"""Bass/Tile MHA kernel for trn2 — builder + host shard/unshard helpers.

Per-core work (8 cores): core c handles batch b=c//2, head-group g=c%2
(8 of 16 heads). Head pairs share 128-partition tiles at bases 0 / 64 so
the K=48 QK^T matmuls land in distinct PE row-groups (free 2x packing),
and the AV matmuls use PSUM column groups 0 / 64 (col packing).

Dataflow (all matmuls bf16 in / fp32 PSUM accumulate):
  qkT[d_h, t]  = w_qk^T x           (lhsT=w_qk tile, rhs=x^T tile)
  V[t, d_v]    = x w_v              (lhsT=x^T tile, rhs=w_v)
  S^T[k, q]    = (K^T)^T Q^T        (K=48 contraction, row-packed pairs)
  P^T          = exp(S^T)           (ScalarE true exp / VectorE fast-exp,
                                     split tunable per kt)
  outT'[d,q],l = (V|1)^T P^T        (ones column gives softmax denoms)
  outT         = outT' * bcast(1/l) + b_v
  y[t, j]      = outT^T w_out       (+ b_out and cross-core sum on host)
"""

import math

import numpy as np
import ml_dtypes

import concourse.bass as bass
import concourse.mybir as mybir
import concourse.tile as tile
from concourse import bacc

F32 = mybir.dt.float32
BF16 = mybir.dt.bfloat16
I16 = mybir.dt.int16
AF = mybir.ActivationFunctionType
OP = mybir.AluOpType

DIM = 768
PH = 48
NP = 4          # head pairs per core
HC = 8          # heads per core
NDT = DIM // 128  # 6 contraction tiles for the projections

# Schraudolph fast-exp in bf16 bit space: bits = round(x*128/ln2 + (127*128 - C))
SCH_A = 128.0 / math.log(2.0)
SCH_C = 4.7
# +0.5: the fp32->int16 convert truncates, this re-centers it to round-nearest
SCH_B = 127.0 * 128.0 - SCH_C + 0.5


def build_kernel(T=2048, dve_slots=frozenset({1, 3, 5, 7, 9, 11, 13}),
                 num_devices=8, debug_taps=False):
    """Returns compiled Bacc module. dve_slots: which of 16 (kt*2+hh)%16
    pipeline slots run fast-exp on VectorE instead of exp on ScalarE."""
    KT = T // 128                 # k-tiles (token tiles)
    QCW = min(512, T)             # q chunk width (one PSUM bank)
    NQG = T // QCW                # q groups, one chunk each

    nc = bacc.Bacc("TRN2", target_bir_lowering=False, debug=False,
                   num_devices=num_devices)
    taps = {}
    if debug_taps:
        taps["qk"] = nc.dram_tensor("tap_qk", (128, NP * 2 * T), BF16, kind="ExternalOutput")
        taps["v"] = nc.dram_tensor("tap_v", (128, KT * HC * 65), BF16, kind="ExternalOutput")
        taps["pt"] = nc.dram_tensor("tap_pt", (128, QCW), BF16, kind="ExternalOutput")
        taps["av"] = nc.dram_tensor("tap_av", (128, QCW), F32, kind="ExternalOutput")
        taps["rbc"] = nc.dram_tensor("tap_rbc", (128, QCW), F32, kind="ExternalOutput")
        taps["outT"] = nc.dram_tensor("tap_outT", (128, NP * T), BF16, kind="ExternalOutput")

    xt_d = nc.dram_tensor("xt", (DIM, T), BF16, kind="ExternalInput")
    wqk_d = nc.dram_tensor("wqk", (DIM, NP * 2 * 128), BF16, kind="ExternalInput")
    wv_d = nc.dram_tensor("wv", (DIM, HC * PH), BF16, kind="ExternalInput")
    wo_d = nc.dram_tensor("wo", (NP * 128, DIM), BF16, kind="ExternalInput")
    bqk_d = nc.dram_tensor("bqk", (128, NP * 2), F32, kind="ExternalInput")
    y_d = nc.dram_tensor("y", (T, DIM), F32, kind="ExternalOutput")

    with tile.TileContext(nc) as tc:
        with (
            tc.tile_pool(name="const", bufs=1) as cpool,
            tc.tile_pool(name="pt", bufs=8) as ptpool,
            tc.tile_pool(name="norm", bufs=2) as npool,
            tc.tile_pool(name="ysb", bufs=2) as ypool,
            tc.tile_pool(name="st", bufs=4, space="PSUM") as stpool,
            tc.tile_pool(name="av", bufs=3, space="PSUM") as avpool,
            tc.tile_pool(name="pj", bufs=1, space="PSUM") as pjpool,
        ):
            # ---- persistent SBUF tensors ----
            xt_sb = cpool.tile([128, NDT, T], BF16, tag="xt")
            wqk_sb = cpool.tile([128, NDT, NP * 2 * 128], BF16, tag="wqk")
            wv_sb = cpool.tile([128, NDT, HC * PH], BF16, tag="wv")
            wo_sb = cpool.tile([128, NP, DIM], BF16, tag="wo")
            bqk_sb = cpool.tile([128, NP * 2], F32, tag="bqk")
            # per-head Q^T/K^T, rows 48-127 zeroed: padding the QK
            # contraction to K=128 keeps every matmul in plain 128x128 mode
            # (mode switches drain the PE and cost ~300ns each)
            qk_sb = cpool.tile([128, HC, 2, T], BF16, tag="qk")
            # V' columns per head: 0 = ones (softmax denominator lands on
            # PSUM row 0 / 64 of the shared bank), 1-48 = V, 49-63 = zero
            v_sb = cpool.tile([128, KT, HC, 64], BF16, tag="v")
            outT_sb = cpool.tile([128, NP, T], BF16, tag="outT")

            # ---- input DMAs ----
            for dt_i in range(NDT):
                nc.sync.dma_start(xt_sb[:, dt_i, :], xt_d[dt_i * 128:(dt_i + 1) * 128, :])
                nc.sync.dma_start(wqk_sb[:, dt_i, :], wqk_d[dt_i * 128:(dt_i + 1) * 128, :])
                nc.sync.dma_start(wv_sb[:, dt_i, :], wv_d[dt_i * 128:(dt_i + 1) * 128, :])
            for p in range(NP):
                nc.sync.dma_start(wo_sb[:, p, :], wo_d[p * 128:(p + 1) * 128, :])
            nc.sync.dma_start(bqk_sb[:], bqk_d[:])

            # ones column for the softmax-denominator trick; zero pads
            nc.gpsimd.memset(v_sb[:, :, :, 0:1], 1.0)
            nc.gpsimd.memset(v_sb[:, :, :, PH + 1:64], 0.0)
            nc.gpsimd.memset(qk_sb[32:64, :, :, :], 0.0)
            nc.gpsimd.memset(qk_sb[64:96, :, :, :], 0.0)
            nc.gpsimd.memset(qk_sb[96:128, :, :, :], 0.0)
            # pad rows (48-63, 112-127) must be finite; 32-aligned starts,
            # the real rows 32-47 / 96-111 are overwritten by normalize
            nc.gpsimd.memset(outT_sb[32:64, :, :], 0.0)
            nc.gpsimd.memset(outT_sb[96:128, :, :], 0.0)

            def qkT_proj(p):
                # qkT[d_h, t] for pair p: rows 0-47 head A dims, 64-111 head B
                for qk in range(2):
                    col0 = (p * 2 + qk) * 128
                    for tcI in range(T // QCW):
                        ps = pjpool.tile([128, QCW], F32, tag="pj")
                        for dt_i in range(NDT):
                            for mh in range(2):
                                nc.tensor.matmul(
                                    ps[mh * 64:(mh + 1) * 64, :],
                                    wqk_sb[:, dt_i, col0 + mh * 64:col0 + (mh + 1) * 64],
                                    xt_sb[:, dt_i, tcI * QCW:(tcI + 1) * QCW],
                                    start=(dt_i == 0), stop=(dt_i == NDT - 1),
                                    skip_group_check=True)
                        tsl = np.s_[tcI * QCW:(tcI + 1) * QCW]
                        nc.scalar.activation(
                            qk_sb[0:PH, p * 2, qk, tsl], ps[0:PH, :],
                            AF.Identity, bias=bqk_sb[0:PH, p * 2 + qk:p * 2 + qk + 1])
                        nc.scalar.activation(
                            qk_sb[0:PH, p * 2 + 1, qk, tsl], ps[64:64 + PH, :],
                            AF.Identity, bias=bqk_sb[64:64 + PH, p * 2 + qk:p * 2 + qk + 1])

            def v_proj():
                for tt in range(KT):
                    # full-bank tile so every pj slot stays bank-aligned
                    psb = pjpool.tile([128, 512], F32, tag="pj")
                    ps = psb[:, 0:HC * PH]
                    for dt_i in range(NDT):
                        for mh in range(2):
                            nc.tensor.matmul(
                                ps[mh * 64:(mh + 1) * 64, :],
                                xt_sb[:, dt_i, tt * 128 + mh * 64:tt * 128 + (mh + 1) * 64],
                                wv_sb[:, dt_i, :],
                                start=(dt_i == 0), stop=(dt_i == NDT - 1),
                                skip_group_check=True)
                    nc.scalar.activation(
                        v_sb[:, tt, :, 1:PH + 1],
                        ps[:].rearrange("p (h d) -> p h d", h=HC),
                        AF.Copy)

            def attention(p, qg):
                cs = np.s_[qg * QCW:(qg + 1) * QCW]
                av = avpool.tile([128, QCW], F32, tag="av", name="av")
                LEAD = 2  # AV trails QK by 2 k-tiles so exp latency hides
                pts = {}

                def qk_emit(kt):
                    for hh in range(2):
                        st = stpool.tile([128, QCW], F32, tag="st")
                        for mh in range(2):
                            nc.tensor.matmul(
                                st[mh * 64:(mh + 1) * 64, :],
                                qk_sb[:, p * 2 + hh, 1,
                                      kt * 128 + mh * 64:kt * 128 + (mh + 1) * 64],
                                qk_sb[:, p * 2 + hh, 0, cs],
                                start=True, stop=True,
                                skip_group_check=True)
                        pt = ptpool.tile([128, QCW], BF16, tag="pt")
                        if (kt * 2 + hh) % 16 in dve_slots:
                            nc.vector.tensor_scalar(
                                pt[:].bitcast(I16), st[:], SCH_A, SCH_B,
                                OP.mult, OP.add)
                        else:
                            nc.scalar.activation(pt[:], st[:], AF.Exp)
                        pts[(kt, hh)] = pt
                        if debug_taps and p == 0 and qg == 0 and kt == 0 and hh == 0:
                            nc.sync.dma_start(taps["pt"][:], pt[:])

                def av_emit(kt):
                    for hh in range(2):
                        nc.tensor.matmul(
                            av[hh * 64:(hh + 1) * 64, :],
                            v_sb[:, kt, p * 2 + hh, :],
                            pts.pop((kt, hh))[:],
                            start=(kt == 0), stop=(kt == KT - 1),
                            skip_group_check=True)

                for kt in range(KT + LEAD):
                    if kt < KT:
                        qk_emit(kt)
                    if kt >= LEAD:
                        av_emit(kt - LEAD)
                # normalize + bias into outT (denominators live in row 64).
                # partition_broadcast is only reliable with base-0 in/out APs,
                # so each head gets its own base-0 recip + broadcast tiles.
                r2a = npool.tile([128, QCW], F32, tag="r2", name="r2a")
                r2b = npool.tile([128, QCW], F32, tag="r2", name="r2b")
                rbca = npool.tile([128, QCW], F32, tag="rbc", name="rbca")
                rbcb = npool.tile([128, QCW], F32, tag="rbc", name="rbcb")
                lra = npool.tile([128, QCW], F32, tag="lr", name="lra")
                lrb = npool.tile([128, QCW], F32, tag="lr", name="lrb")
                nc.scalar.copy(lra[0:1, :], av[0:1, :])
                nc.scalar.copy(lrb[0:1, :], av[64:65, :])
                nc.vector.reciprocal_approx_fast(r2a[0:1, :], lra[0:1, :])
                nc.vector.reciprocal_approx_fast(r2b[0:1, :], lrb[0:1, :])
                nc.gpsimd.partition_broadcast(rbca[0:PH + 1, :], r2a[0:1, :])
                nc.gpsimd.partition_broadcast(rbcb[0:PH + 1, :], r2b[0:1, :])
                nc.vector.tensor_mul(outT_sb[0:PH + 1, p, cs],
                                     av[0:PH + 1, :], rbca[0:PH + 1, :])
                nc.vector.tensor_mul(outT_sb[64:64 + PH + 1, p, cs],
                                     av[64:64 + PH + 1, :], rbcb[0:PH + 1, :])
                if debug_taps and p == 0 and qg == 0:
                    avs = npool.tile([128, QCW], F32, tag="avs")
                    nc.vector.tensor_copy(avs[0:65, :], av[0:65, :])
                    nc.sync.dma_start(taps["av"][0:65, :], avs[0:65, :])
                    nc.sync.dma_start(taps["rbc"][0:PH, :], rbca[0:PH, :])
                    nc.sync.dma_start(taps["rbc"][64:64 + PH, :], rbcb[0:PH, :])

            def final_proj(qg):
                for tt in range(QCW // 128):
                    t0 = qg * QCW + tt * 128
                    ysb = ypool.tile([128, DIM], F32, tag="ysb")
                    for jc in range(2):
                        js = np.s_[jc * 384:(jc + 1) * 384]
                        psb = pjpool.tile([128, 512], F32, tag="pj", name=f"yp{jc}")
                        ps = psb[:, 0:384]
                        for p in range(NP):
                            for mh in range(2):
                                nc.tensor.matmul(
                                    ps[mh * 64:(mh + 1) * 64, :],
                                    outT_sb[:, p, t0 + mh * 64:t0 + (mh + 1) * 64],
                                    wo_sb[:, p, js],
                                    start=(p == 0), stop=(p == NP - 1),
                                    skip_group_check=True)
                        if (tt + jc) % 2 == 0:
                            nc.scalar.copy(ysb[:, js], ps[:])
                        else:
                            nc.vector.tensor_copy(ysb[:, js], ps[:])
                    nc.sync.dma_start(y_d[t0:t0 + 128, :], ysb[:])

            # ---- emission order (scheduling priority) ----
            qkT_proj(0)
            v_proj()
            for qg in range(NQG):
                for p in range(NP):
                    if qg == 0 and p + 1 < NP:
                        qkT_proj(p + 1)
                    attention(p, qg)
                final_proj(qg)

            if debug_taps:
                nc.sync.dma_start(
                    taps["qk"][:], qk_sb[:].rearrange("p a b t -> p (a b t)"))
                nc.sync.dma_start(
                    taps["v"][:], v_sb[:].rearrange("p a b c -> p (a b c)"))
                nc.sync.dma_start(
                    taps["outT"][:], outT_sb[:].rearrange("p a t -> p (a t)"))

    nc.compile()
    return nc


# ---------------- host-side sharding ----------------

def host_prep(x, w_in, b_in, w_out, T=2048):
    """Full inputs -> list of 8 per-core input dicts."""
    scale = 1.0 / math.sqrt(PH)
    wr = np.asarray(w_in).reshape(DIM, 16, 3, PH)
    br = np.asarray(b_in).reshape(16, 3, PH)
    wog = np.asarray(w_out)  # (768, 768), row dv = h*48+d
    in_maps = []
    for c in range(8):
        b, g = divmod(c, 2)
        wqk = np.zeros((DIM, NP * 2 * 128), np.float32)
        bqk = np.zeros((128, NP * 2), np.float32)
        wv = np.zeros((DIM, HC * PH), np.float32)
        wo = np.zeros((NP * 128, DIM), np.float32)
        for p in range(NP):
            for hh, base in ((0, 0), (1, 64)):
                gh = g * 8 + p * 2 + hh
                wqk[:, (p * 2) * 128 + base:(p * 2) * 128 + base + PH] = wr[:, gh, 0] * scale
                wqk[:, (p * 2 + 1) * 128 + base:(p * 2 + 1) * 128 + base + PH] = wr[:, gh, 1]
                bqk[base:base + PH, p * 2] = br[gh, 0] * scale
                bqk[base:base + PH, p * 2 + 1] = br[gh, 1]
                wv[:, (p * 2 + hh) * PH:(p * 2 + hh + 1) * PH] = wr[:, gh, 2]
                wo[p * 128 + base + 1:p * 128 + base + 1 + PH, :] = wog[gh * PH:(gh + 1) * PH, :]
        in_maps.append({
            "xt": np.ascontiguousarray(np.asarray(x)[b].T).astype(ml_dtypes.bfloat16),
            "wqk": wqk.astype(ml_dtypes.bfloat16),
            "wv": wv.astype(ml_dtypes.bfloat16),
            "wo": wo.astype(ml_dtypes.bfloat16),
            "bqk": bqk,
        })
    return in_maps


def host_post(results, b_out, b_in, w_out, B=4, T=2048):
    # the V bias contributes bv @ w_out, a per-column constant: add on host
    bv_all = np.asarray(b_in).reshape(16, 3, PH)[:, 2, :].reshape(DIM)
    const = np.asarray(b_out) + bv_all @ np.asarray(w_out)
    out = np.empty((B, T, DIM), np.float32)
    for b in range(B):
        out[b] = results[2 * b]["y"] + results[2 * b + 1]["y"] + const[None, :]
    return out


# ---------------- self-contained kernel() entry point ----------------

_CACHED = {}


def _get_nc():
    if "nc" not in _CACHED:
        _CACHED["nc"] = build_kernel(T=2048, num_devices=8)
    return _CACHED["nc"]


def kernel(x, w_in, b_in, w_out, b_out):
    """Full-input MHA forward on 8 NeuronCores.

    x: (4, 2048, 768) f32; w_in: (768, 2304); b_in: (2304,);
    w_out: (768, 768); b_out: (768,). Returns (4, 2048, 768) f32.
    """
    from concourse.bass_utils import run_bass_kernel_spmd

    x = np.asarray(x, np.float32)
    w_in = np.asarray(w_in, np.float32)
    b_in = np.asarray(b_in, np.float32)
    w_out = np.asarray(w_out, np.float32)
    b_out = np.asarray(b_out, np.float32)

    nc = _get_nc()
    in_maps = host_prep(x, w_in, b_in, w_out, T=2048)
    res = run_bass_kernel_spmd(nc, in_maps, core_ids=list(range(8)))
    return host_post(res.results, b_out, b_in, w_out, B=4, T=2048)

